# revision 27
# baseline (speedup 1.0000x reference)
"""CTC aligner kernel for Trainium2 (8 NeuronCores, data-parallel over batch).

The reference's forward/backward recursions collapse: in each scan step the
logsumexp factors out (f_prev has no s-dependence in the added term), so with
q[t] = logits[t] + trans[t-1] (+b2), q[0] = f0, q[T] = 0:
    alignments[t] = softmax_s(q[t] + q[t+1])
The only heavy compute left is the pair MLP (trans), a batched matmul.

Design v2 (per core, 4 examples as 2 partition-packed pairs; ~41.4us HW
vs 44.3us for the previous t-partitioned MM2 design):
- MM1 (fp8 DoubleRow, K=256 x2 accumulated in PSUM; the kc time-shift
  implements concat(fe[t+1], fe[t]) for free) -> relu+b1 alternating on
  Scalar/Vector -> hidden stored fp8 BYTE-INTERLEAVED [128, t, 2] so it can
  feed MM2's DoubleRow moving operand directly. MM1 runs at the fp8 PE
  roofline (215ns per 512-wide DR matmul at 2.4GHz).
- MM2 is weight-stationary: lhsT = W2 (fp8 DR-packed [128, 2, 64]), rhs =
  hidden chunk [128, 2, 512], out = trans^T [64*parity+s, t] in PSUM —
  4-5 wide matmuls per example instead of 34 small LDWEIGHTS-bound ones.
  The pair's even example uses DoubleRow into partitions 0:64 (DR output
  occupies all 128 PE columns, so dst partitions >=64 are not encodable);
  the odd example uses two accumulated non-DR fp8 matmuls at PE tile
  position col 64 into partitions 64:128 of the same bank, so every
  epilogue op runs 128 partitions wide.
- Softmax via exp-product in [s, t] layout: e[.,1+u] = exp(TR[u]), e[.,0]=1,
  P2[., t] = e[., t] * e[., t+1] * eL[., t] with eL = exp(Lsum) shipped fp16
  from host (edges fold f0 / q[T]=0 / b2). exp on Scalar per 512-chunk,
  shift-product split GpSimd (early chunks; 0.42-efficiency is fine when it
  overlaps PE work) / Vector (tail chunks), eL-product on Vector.
  Unnormalized P2 ships fp16; host divides by row sums.
- PE order interleaves mm2(pair, c) between mm1 groups one-to-two chunks
  behind the producing example. Relus alternate S/V *within* each example
  (a blocked split serializes one engine and stalls the pair's first mm2).
  Per-example input DMAs + a split first chunk keep mm1 off the DMA
  critical path; per-chunk output DMAs keep the final transfer small; the
  last example's last chunk splits relu/mm2/exp/mul/DMA at half-chunk
  grain to shorten the serial tail. Junk warmup matmuls ramp the PE clock
  (0.65 -> 2.4GHz takes ~3.5us of sustained activity) while the head DMAs
  land. Remaining fixed costs: ~6us framework preamble (excluded from the
  measured window) and ~10us NEFF wrapper teardown (counted, runs at 4/8
  activity-throttle duty).
"""

import numpy as np
import ml_dtypes

B, T, S, H = 32, 2000, 63, 256
NCORES = 8
BPC = B // NCORES  # examples per core
NPAIR = BPC // 2   # partition-packed pairs per core
TPAD = 2048
CW = 512           # chunk width (time)
NCHUNK = TPAD // CW
WBLOB = 580        # weights blob units: 512 (W1 fp8) + 64 (W2 fp8 DR) + 4 (b1 f32)

_built = {}


def _build():
    from concourse import mybir, tile, bacc

    f32 = mybir.dt.float32
    bf16 = mybir.dt.bfloat16
    fp16 = mybir.dt.float16
    fp8 = mybir.dt.float8e4
    Alu = mybir.AluOpType
    Act = mybir.ActivationFunctionType
    DR = mybir.MatmulPerfMode.DoubleRow

    nc = bacc.Bacc("TRN2", target_bir_lowering=False, debug=False,
                   num_devices=NCORES)

    fin = nc.dram_tensor("fin", [BPC, 128, TPAD], bf16,
                         kind="ExternalInput").ap()
    elin = nc.dram_tensor("elin", [NPAIR, 128, TPAD], fp16,
                          kind="ExternalInput").ap()
    wblob = nc.dram_tensor("wblob", [128, WBLOB], bf16,
                           kind="ExternalInput").ap()
    out_t = nc.dram_tensor("out_t", [NPAIR, 128, TPAD], fp16,
                           kind="ExternalOutput").ap()

    # MM1 time chunks over the 1999 pair indices
    CHUNKS = [(0, 512), (512, 512), (1024, 512), (1536, 463)]

    with tile.TileContext(nc) as tc:
        with (
            tc.tile_pool(name="sb", bufs=1) as sbp,
            tc.tile_pool(name="mm1ps", bufs=6, space="PSUM") as mm1p,
            tc.tile_pool(name="mm2ps", bufs=2, space="PSUM") as mm2p,
        ):
            # PE warmup: junk matmuls ramp the clock while the head DMAs land
            wsrc = sbp.tile([128, 512], bf16, tag="wsrc")
            nc.gpsimd.memset(wsrc[:], 0.0)
            wact = sbp.tile([128, 1], f32, tag="wact")
            nc.scalar.activation(wact[:], wsrc[:, 0:1], Act.Exp)
            wps = mm1p.tile([128, 512], f32, tag="mm1")
            for i in range(8):
                nc.tensor.matmul(wps[:], wsrc[:, 0:128], wsrc[:],
                                 start=True, stop=True)

            # input DMAs: first example's first chunk ASAP, then per-example
            # pieces so each example's MM1 gates only on its own transfer
            fin0 = sbp.tile([128, TPAD], bf16, tag="fin0")
            nc.sync.dma_start(fin0[:, 0:514], fin[0, :, 0:514])
            wsb = sbp.tile([128, WBLOB], bf16, tag="wsb")
            nc.sync.dma_start(wsb[:], wblob)
            nc.sync.dma_start(fin0[:, 514:1026], fin[0, :, 514:1026])
            nc.sync.dma_start(fin0[:, 1026:TPAD], fin[0, :, 1026:TPAD])
            finr = sbp.tile([128, BPC - 1, TPAD], bf16, tag="finr")
            elt = sbp.tile([128, NPAIR, TPAD], fp16, tag="el")
            nc.sync.dma_start(finr[:, 0], fin[1])
            nc.sync.dma_start(finr[:, 1], fin[2])
            nc.sync.dma_start(elt[:], elin[:].rearrange("b p t -> p b t"))
            nc.sync.dma_start(finr[:, 2], fin[3])
            fins = [fin0] + [finr[:, b - 1] for b in range(1, BPC)]

            w1sb = wsb[:, 0:512].bitcast(fp8).rearrange(
                "p (kc i j) -> p kc i j", kc=2, i=2)
            w2sb = wsb[:, 512:576].bitcast(fp8).rearrange(
                "p (kt s) -> p kt s", kt=2)
            b1sb = wsb[:, 576:580].bitcast(f32)

            # hidden: fp8, k-halves byte-interleaved -> [128, t, 2]
            hids = []
            for b in range(BPC):
                hid = sbp.tile([128, TPAD, 2], fp8, tag=f"hid{b}")
                nc.gpsimd.memset(hid[:, 1999:TPAD, :], 0.0)
                hids.append(hid)
            # e[., 0] = 1 (the q[-1] edge); e[., 1+u] = exp(TR[u])
            es, ps_, p2s = [], [], []
            for p in range(NPAIR):
                e = sbp.tile([128, TPAD + 4], fp16, tag=f"e{p}")
                nc.gpsimd.memset(e[:, 0:1], 1.0)
                es.append(e)
                ps_.append(sbp.tile([128, TPAD], fp16, tag=f"p{p}",
                                    name=f"p{p}"))
                p2s.append(sbp.tile([128, TPAD], fp16, tag=f"p2{p}",
                                    name=f"p2{p}"))

            relu_flip = [0]

            # per-relu engine schedule: alternate S/V inside each example so
            # neither engine builds a serial backlog that stalls the first
            # mm2 of a pair; e2 skews 5:3 toward the faster scalar engine
            RELU_ENG = ("svsvsvsv" "svsvsvsv" "svssvssv" "svsvsvsv")

            def emit_relu(dst, src, j, engine=None, high=False):
                on_scalar = (RELU_ENG[relu_flip[0]] == "s") if engine is None \
                    else engine == "s"
                relu_flip[0] += 1
                ctx = tc.high_priority(offset=20) if high else None
                if ctx is not None:
                    ctx.__enter__()
                try:
                    if on_scalar:
                        nc.scalar.activation(dst, src, Act.Relu,
                                             bias=b1sb[:, j:j + 1])
                    else:
                        nc.vector.tensor_scalar(
                            dst, src, b1sb[:, j:j + 1], 0.0,
                            op0=Alu.add, op1=Alu.max)
                finally:
                    if ctx is not None:
                        ctx.__exit__(None, None, None)

            def emit_mm1_group(b, ci, split_relu=False):
                t0, w = CHUNKS[ci]
                ff8 = fins[b].bitcast(fp8)
                hid = hids[b]
                pss = []
                for j in range(2):
                    ps = mm1p.tile([128, 512], f32, tag="mm1")
                    for kc in range(2):
                        toff = t0 + 1 if kc == 0 else t0
                        rhs = ff8[:, 2 * toff:2 * (toff + w)].rearrange(
                            "p (t i) -> p i t", i=2)
                        nc.tensor.matmul(
                            ps[:, :w],
                            w1sb[:, kc, :, 128 * j:128 * j + 128],
                            rhs,
                            start=(kc == 0), stop=(kc == 1),
                            perf_mode=DR,
                        )
                    if not split_relu:
                        emit_relu(hid[:, t0:t0 + w, j], ps[:, :w], j)
                    pss.append(ps)
                if split_relu:
                    # tail: half-granularity relus, S/V in parallel per half
                    hb = w // 2
                    emit_relu(hid[:, t0:t0 + hb, 0], pss[0][:, 0:hb], 0, "s")
                    emit_relu(hid[:, t0:t0 + hb, 1], pss[1][:, 0:hb], 1, "v")
                    emit_relu(hid[:, t0 + hb:t0 + w, 0], pss[0][:, hb:w], 0,
                              "s")
                    emit_relu(hid[:, t0 + hb:t0 + w, 1], pss[1][:, hb:w], 1,
                              "v")

            def emit_mm2(pr, c, split_tail=False):
                t0 = c * CW
                tr = mm2p.tile([128, CW], f32, tag="mm2")
                e, p, p2 = es[pr], ps_[pr], p2s[pr]
                el = elt[:, pr]
                # mul1 on GpSimd only where it overlaps plenty of PE work;
                # the tail chunks stay on the faster Vector engine
                pool_mul1 = (pr, c) in ((0, 0), (0, 1), (0, 2), (0, 3),
                                        (1, 0), (1, 1))
                # tail halves align with the split relu of mm1 (3,3)
                halves = ((0, 231), (231, 281)) if split_tail else ((0, CW),)
                for (h0, hw) in halves:
                    # even example: fp8 DoubleRow -> partitions 0:64 (DR
                    # output uses all 128 PE columns, so it can only target
                    # cols 0:128 -> dst partitions 0:64)
                    hid = hids[2 * pr]
                    rhs = hid[:, t0 + h0:t0 + h0 + hw, :].rearrange(
                        "p t i -> p i t")
                    nc.tensor.matmul(tr[0:64, h0:h0 + hw], w2sb[:], rhs,
                                     start=True, stop=True, perf_mode=DR)
                    # odd example: two accumulated non-DR fp8 matmuls at PE
                    # tile position col 64 -> partitions 64:128 of same bank
                    hid = hids[2 * pr + 1]
                    for kt in range(2):
                        nc.tensor.matmul(
                            tr[64:128, h0:h0 + hw], w2sb[:, kt],
                            hid[:, t0 + h0:t0 + h0 + hw, kt],
                            start=(kt == 0), stop=(kt == 1),
                        )
                    a = t0 + h0
                    nc.scalar.activation(e[:, 1 + a:1 + a + hw],
                                         tr[:, h0:h0 + hw], Act.Exp)
                    if pool_mul1:
                        nc.gpsimd.tensor_mul(p[:, a:a + hw], e[:, a:a + hw],
                                             e[:, 1 + a:1 + a + hw])
                    else:
                        nc.vector.tensor_mul(p[:, a:a + hw], e[:, a:a + hw],
                                             e[:, 1 + a:1 + a + hw])
                    nc.vector.tensor_mul(p2[:, a:a + hw], p[:, a:a + hw],
                                         el[:, a:a + hw])
                    if split_tail:
                        nc.sync.dma_start(
                            out_t[pr, :, a:a + hw], p2[:, a:a + hw])

            # PE schedule: mm1 groups in (example, chunk) order with mm2
            # jobs interleaved behind the producing example (pair0 lags two
            # groups so the slow-clock early relus stay off the PE critical
            # path); per-chunk out DMAs keep the final transfer small
            mm2_after = {
                (1, 2): (0, 0), (1, 3): (0, 1), (2, 0): (0, 2),
                (2, 1): (0, 3),
                (3, 1): (1, 0), (3, 2): (1, 1), (3, 3): (1, 2),
            }
            for b in range(BPC):
                for ci in range(NCHUNK):
                    emit_mm1_group(b, ci, split_relu=(b, ci) == (3, 3))
                    job = mm2_after.get((b, ci))
                    if job is not None:
                        emit_mm2(*job)
                        pr, c = job
                        nc.sync.dma_start(out_t[pr, :, c * CW:(c + 1) * CW],
                                          p2s[pr][:, c * CW:(c + 1) * CW])
            emit_mm2(1, 3, split_tail=True)
    nc.compile()
    return nc


def _get_nc():
    if "nc" not in _built:
        _built["nc"] = _build()
    return _built["nc"]


def prep_in_maps(ctc_logits, frame_embeddings, keyword_tokens, W1, b1, W2, b2):
    fp8np = ml_dtypes.float8_e4m3
    bf16np = ml_dtypes.bfloat16

    ctc_logits = np.asarray(ctc_logits, dtype=np.float32)
    frame_embeddings = np.asarray(frame_embeddings, dtype=np.float32)
    keyword_tokens = np.asarray(keyword_tokens)
    W1 = np.asarray(W1, dtype=np.float32)
    b1 = np.asarray(b1, dtype=np.float32)
    W2 = np.asarray(W2, dtype=np.float32)
    b2 = np.asarray(b2, dtype=np.float32)

    # f0[b, s] = (s == 0) * log_softmax(logits[b, 0])[kw[b, 0]]
    lg0 = ctc_logits[:, 0, :].astype(np.float64)
    m = lg0.max(axis=-1)
    lse = m + np.log(np.exp(lg0 - m[:, None]).sum(axis=-1))
    kw0 = keyword_tokens[:, 0].astype(np.int64)
    f0 = np.zeros((B, S), np.float64)
    f0[:, 0] = lg0[np.arange(B), kw0] - lse

    # frames -> fp8 pairs (h, h+128) as 2-byte units, pre-transposed to
    # [b, p, t] so the device DMA is a straight contiguous copy
    f8 = np.zeros((B, TPAD, H), fp8np)
    f8[:, :T] = frame_embeddings.astype(fp8np)
    lo = f8[:, :, 0:128].view(np.uint8).astype(np.uint16)
    hi = f8[:, :, 128:256].view(np.uint8).astype(np.uint16)
    pk = np.ascontiguousarray((lo | (hi << 8)).transpose(0, 2, 1))  # (B,128,T)

    # Lsum[t] = score minus the trans terms:
    #   t = 0:        f0 + L[1] + b2
    #   1..T-2:       L[t] + L[t+1] + 2 b2
    #   t = T-1:      L[T-1] + b2
    #   t >= T:       -inf (eL = 0; pad cols, discarded on host)
    Lf = ctc_logits.astype(np.float64)
    lsum = np.zeros((B, TPAD, S), np.float64)
    lsum[:, 1:T - 1] = Lf[:, 1:T - 1] + Lf[:, 2:T] + 2.0 * b2
    lsum[:, 0] = f0 + Lf[:, 1] + b2
    lsum[:, T - 1] = Lf[:, T - 1] + b2
    el = np.zeros((B, TPAD, S), np.float64)
    el[:, :T] = np.exp(lsum[:, :T])

    # pair-packed eL: [pair, 64*parity + s, t]; rows 63/127 stay 0
    elp = np.zeros((B // 2, 128, TPAD), np.float16)
    elp[:, 0:S] = el[0::2].transpose(0, 2, 1)
    elp[:, 64:64 + S] = el[1::2].transpose(0, 2, 1)

    # weights blob: W1dr fp8 (1024B) | W2 DR-packed fp8 (128B) | b1 f32 (8B)
    # W1dr[p][kc, i, j] = W1[256*kc + 128*i + p, j]
    w1dr = np.ascontiguousarray(
        W1.reshape(2, 2, 128, H).transpose(2, 0, 1, 3))  # (128, kc, i, j)
    w1bytes = w1dr.astype(fp8np).reshape(128, 1024).view(np.uint8)
    # w2dr[p][kt, s] = W2[128*kt + p, s], s padded to 64
    w2dr = np.zeros((128, 2, 64), fp8np)
    w2dr[:, 0, 0:S] = W2[0:128].astype(fp8np)
    w2dr[:, 1, 0:S] = W2[128:256].astype(fp8np)
    w2bytes = w2dr.reshape(128, 128).view(np.uint8)
    b1p = np.stack([b1[0:128], b1[128:256]], axis=1)  # (128, 2)
    b1bytes = np.ascontiguousarray(b1p.astype(np.float32)).view(
        np.uint8).reshape(128, 8)
    wb = np.concatenate([w1bytes, w2bytes, b1bytes], axis=1)  # (128, 1160)
    wb = np.ascontiguousarray(wb).view(np.uint16)  # (128, 580)

    in_maps = []
    for c in range(NCORES):
        sl = slice(BPC * c, BPC * (c + 1))
        slp = slice((BPC // 2) * c, (BPC // 2) * (c + 1))
        in_maps.append({
            "fin": np.ascontiguousarray(pk[sl]).view(bf16np),
            "elin": np.ascontiguousarray(elp[slp]),
            "wblob": wb.view(bf16np),
        })
    return in_maps


def untile_out(res_out):
    # res_out: (NPAIR, 128, TPAD) fp16; partition = 64*parity + s
    r = res_out.reshape(NPAIR, 2, 64, TPAD)[:, :, 0:S, :T]  # (pr, par, s, t)
    return r.transpose(0, 1, 3, 2).reshape(BPC, T, S)


def kernel(ctc_logits, frame_embeddings, keyword_tokens, W1, b1, W2, b2):
    from concourse.bass_utils import run_bass_kernel_spmd

    in_maps = prep_in_maps(ctc_logits, frame_embeddings, keyword_tokens,
                           W1, b1, W2, b2)
    nc = _get_nc()
    res = run_bass_kernel_spmd(nc, in_maps, list(range(NCORES)))
    out = np.concatenate([untile_out(np.asarray(res.results[c]["out_t"]))
                          for c in range(NCORES)], axis=0).astype(np.float32)
    out /= out.sum(axis=-1, keepdims=True)
    return np.ascontiguousarray(out)
